# revision 14
# baseline (speedup 1.0000x reference)
"""Trainium2 Bass kernel for nn_C2f_DualModal_MoE (C2f block with top-1 MoE routing).

Strategy (data-parallel over batch, 4 samples per core on 8 cores):
  - cv1 (1x1 conv 256->256 + SiLU) as f32r matmuls over 400-pixel tiles;
    the `feat` half is written into a zero-padded [82x82] spatial layout so
    the 3x3 convs become 9 shift-offset matmuls. The global-average-pool for
    the router comes free via the activation accum_out.
  - Router: tiny f32 matmul + softmax on-chip; the top-1 selection is turned
    into a one-hot vector (no control flow), which selects the routed expert's
    weights via 3 vector ops (Wsel = sum_e onehot[e] * We[e]); since top-1,
    conv(feat, Wsel) == conv(feat, We[argmax]).
  - shared + routed 3x3 convs (SiLU), moe = shared + gate * routed.
  - cv2 (1x1 conv 384->256 + SiLU) fused per tile from (a, feat, moe) without
    materializing the concat; routed-conv and cv2 are software-pipelined by
    one tile.
All matmuls use float32r (full-rate PE); everything else f32.
"""

import numpy as np

import concourse.bass as bass
import concourse.bacc as bacc
import concourse.tile as tile
from concourse import mybir
from concourse.bass_utils import run_bass_kernel_spmd

# Problem constants (hardcoded per contract)
B, C1, C2 = 32, 256, 256
H = W = 80
CH = 128
NE = 3
NCORES = 8
BPC = B // NCORES          # samples per core = 4
NPIX = H * W               # 6400
PADW = W + 2               # 82
PADH = H + 2               # 82
RPT = 5                    # rows per pixel tile
TN = RPT * W               # 400 pixels per tile
NT = H // RPT              # 16 tiles
TAPS = [(dy, dx) for dy in range(3) for dx in range(3)]

f32 = mybir.dt.float32
f32r = mybir.dt.float32r


def _emit(nc, tc, ctx, reps=1, sim_compat=False):
    AX = mybir.AxisListType
    OP = mybir.AluOpType
    AF = mybir.ActivationFunctionType

    x_d = nc.dram_tensor("x", [BPC, 2, CH, NPIX], f32r, kind="ExternalInput").ap()
    w1_d = nc.dram_tensor("w1t", [2, CH, 2 * CH], f32r, kind="ExternalInput").ap()
    b1_d = nc.dram_tensor("b1r", [2, CH], f32, kind="ExternalInput").ap()
    wr_d = nc.dram_tensor("wrs", [CH, NE], f32, kind="ExternalInput").ap()
    br_d = nc.dram_tensor("brr", [1, NE], f32, kind="ExternalInput").ap()
    ws_d = nc.dram_tensor("wst", [CH, 9 * CH], f32r, kind="ExternalInput").ap()
    bs_d = nc.dram_tensor("bsr", [CH, 1], f32, kind="ExternalInput").ap()
    we_d = nc.dram_tensor("wet", [NE, CH, 9 * CH], f32, kind="ExternalInput").ap()
    be_d = nc.dram_tensor("ber", [CH, NE], f32, kind="ExternalInput").ap()
    w2_d = nc.dram_tensor("w2t", [3, CH, C2], f32r, kind="ExternalInput").ap()
    b2_d = nc.dram_tensor("b2r", [2, CH], f32, kind="ExternalInput").ap()
    y_d = nc.dram_tensor("y", [BPC, 2, CH, NPIX], f32, kind="ExternalOutput").ap()

    wpool = ctx.enter_context(tc.tile_pool(name="weights", bufs=1))
    ppool = ctx.enter_context(tc.tile_pool(name="persist", bufs=1))
    xpool = ctx.enter_context(tc.tile_pool(name="xin", bufs=3))
    opool = ctx.enter_context(tc.tile_pool(name="oout", bufs=4))
    rpool = ctx.enter_context(tc.tile_pool(name="rtile", bufs=2))
    spool = ctx.enter_context(tc.tile_pool(name="small", bufs=2))
    selpool = ctx.enter_context(tc.tile_pool(name="sel", bufs=1))
    psum = ctx.enter_context(tc.tile_pool(name="psum", bufs=6, space="PSUM"))
    psumS = ctx.enter_context(tc.tile_pool(name="psumS", bufs=1, space="PSUM"))

    # ---- load weights into SBUF (resident) ----
    w1_sb = wpool.tile([CH, 2 * 2 * CH], f32r)
    for k in range(2):
        nc.sync.dma_start(w1_sb[:, k * 256:(k + 1) * 256], w1_d[k])
    ws_sb = wpool.tile([CH, 9 * CH], f32r)
    nc.sync.dma_start(ws_sb[:], ws_d)
    we_sb = wpool.tile([CH, NE * 9 * CH], f32)
    for e in range(NE):
        nc.sync.dma_start(we_sb[:, e * 1152:(e + 1) * 1152], we_d[e])
    w2_sb = wpool.tile([CH, 3 * C2], f32r)
    for k in range(3):
        nc.sync.dma_start(w2_sb[:, k * 256:(k + 1) * 256], w2_d[k])
    wr_sb = wpool.tile([CH, NE], f32)
    nc.sync.dma_start(wr_sb[:], wr_d)
    br_sb = wpool.tile([1, NE], f32)
    nc.sync.dma_start(br_sb[:], br_d)
    bs_sb = wpool.tile([CH, 1], f32)
    nc.sync.dma_start(bs_sb[:], bs_d)
    be_sb = wpool.tile([CH, NE], f32)
    nc.sync.dma_start(be_sb[:], be_d)
    b1_sb = wpool.tile([CH, 2], f32)
    for k in range(2):
        nc.sync.dma_start(b1_sb[:, k:k + 1], b1_d[k])
    b2_sb = wpool.tile([CH, 2], f32)
    for k in range(2):
        nc.sync.dma_start(b2_sb[:, k:k + 1], b2_d[k])
    ones_sb = wpool.tile([1, CH], f32)
    nc.vector.memset(ones_sb[:], 1.0)

    # ---- persistent per-sample working buffers ----
    fp = ppool.tile([CH, PADH * PADW], f32r)
    # zero once: borders stay zero forever (bitcast: memset lacks f32r support)
    nc.vector.memset(fp[:].bitcast(f32), 0.0)
    fp3 = fp[:].rearrange("p (r c) -> p r c", c=PADW)
    a_sb = ppool.tile([CH, NPIX], f32r)
    sh_sb = ppool.tile([CH, NPIX], f32)
    moe_sb = ppool.tile([CH, NPIX], f32r)

    tmpool = ctx.enter_context(tc.tile_pool(name="silutmp", bufs=2)) if sim_compat else None

    def act_silu(out_ap, ps_ap, bias_ap, accum_ap=None):
        """SiLU from PSUM -> SBUF. On HW, one ACT instruction (with optional
        free GAP accumulation). CoreSim lacks Silu, so sim_compat emulates via
        Sigmoid + (ps+bias)*sig, and computes the accumulation separately."""
        if not sim_compat:
            if accum_ap is not None:
                nc.scalar.activation(out_ap, ps_ap, AF.Silu, bias=bias_ap,
                                     scale=1.0, accum_out=accum_ap)
            else:
                nc.scalar.activation(out_ap, ps_ap, AF.Silu, bias=bias_ap,
                                     scale=1.0)
            return
        tmp = tmpool.tile([CH, TN], f32, tag="sigmoid_tmp")
        ps2d = ps_ap if len(ps_ap.shape) == 2 else ps_ap.rearrange("p r c -> p (r c)")
        nc.scalar.activation(tmp[:], ps2d, AF.Sigmoid, bias=bias_ap, scale=1.0)
        tmpv = tmp[:] if len(out_ap.shape) == 2 else tmp[:].rearrange(
            "p (r c) -> p r c", c=out_ap.shape[-1])
        nc.vector.scalar_tensor_tensor(out_ap, ps_ap, bias_ap, tmpv,
                                       op0=OP.add, op1=OP.mult)
        if accum_ap is not None:
            axis = mybir.AxisListType.X if len(out_ap.shape) == 2 else mybir.AxisListType.XY
            nc.vector.reduce_sum(accum_ap, out_ap, axis=axis)

    def conv_tile_matmuls(ps, wsb, i):
        for t, (dy, dx) in enumerate(TAPS):
            rhs = fp3[:, i * RPT + dy: i * RPT + dy + RPT, dx: dx + W]
            nc.tensor.matmul(
                ps[:],
                wsb[:, t * CH:(t + 1) * CH],
                rhs,
                start=(t == 0),
                stop=(t == 8),
            )

    def _body():
        for b in range(BPC):
            # ---- cv1: x -> (a, feat) with SiLU; GAP accumulated for free ----
            gap_sb = spool.tile([CH, NT], f32, tag="gap")
            for i in range(NT):
                xt0 = xpool.tile([CH, TN], f32r, tag="xt0")
                nc.sync.dma_start(xt0[:], x_d[b, 0, :, i * TN:(i + 1) * TN])
                xt1 = xpool.tile([CH, TN], f32r, tag="xt1")
                nc.sync.dma_start(xt1[:], x_d[b, 1, :, i * TN:(i + 1) * TN])
                ps_a = psum.tile([CH, TN], f32, tag="ps")
                nc.tensor.matmul(ps_a[:], w1_sb[:, 0:128],
                                 xt0[:], start=True, stop=False)
                nc.tensor.matmul(ps_a[:], w1_sb[:, 256:384],
                                 xt1[:], start=False, stop=True)
                ps_f = psum.tile([CH, TN], f32, tag="ps")
                nc.tensor.matmul(ps_f[:], w1_sb[:, 128:256],
                                 xt0[:], start=True, stop=False)
                nc.tensor.matmul(ps_f[:], w1_sb[:, 384:512],
                                 xt1[:], start=False, stop=True)
                act_silu(a_sb[:, i * TN:(i + 1) * TN], ps_a[:], b1_sb[:, 0:1])
                act_silu(
                    fp3[:, 1 + i * RPT: 1 + (i + 1) * RPT, 1:1 + W],
                    ps_f[:].rearrange("p (r c) -> p r c", c=W),
                    b1_sb[:, 1:2],
                    accum_ap=gap_sb[:, i:i + 1],
                )

            # ---- router: logits -> softmax -> top-1 one-hot + gate ----
            pooled = spool.tile([CH, 1], f32, tag="pooled")
            nc.vector.reduce_sum(pooled[:], gap_sb[:], axis=AX.X)
            ps_l = psumS.tile([1, NE], f32, tag="psl")
            # wr is pre-scaled by 1/NPIX on the host, so sums (not means) work.
            nc.tensor.matmul(ps_l[:], pooled[:], wr_sb[:], start=True, stop=True)
            logits = spool.tile([1, NE], f32, tag="logits")
            nc.vector.tensor_add(logits[:], ps_l[:], br_sb[:])
            m_sb = spool.tile([1, 1], f32, tag="m")
            nc.vector.reduce_max(m_sb[:], logits[:], axis=AX.X)
            negm = spool.tile([1, 1], f32, tag="negm")
            nc.vector.tensor_scalar_mul(negm[:], m_sb[:], -1.0)
            e_sb = spool.tile([1, NE], f32, tag="esb")
            nc.scalar.activation(e_sb[:], logits[:], AF.Exp, bias=negm[:], scale=1.0)
            s_sb = spool.tile([1, 1], f32, tag="ssb")
            nc.vector.reduce_sum(s_sb[:], e_sb[:], axis=AX.X)
            wgt = spool.tile([1, 1], f32, tag="wgt")
            nc.vector.reciprocal(wgt[:], s_sb[:])
            oh = spool.tile([1, NE], f32, tag="oh")
            nc.vector.tensor_scalar(oh[:], logits[:], m_sb[:], None, op0=OP.is_ge)
            bc = spool.tile([1, NE + 1], f32, tag="bc")
            nc.vector.tensor_copy(bc[:, 0:NE], oh[:])
            nc.vector.tensor_copy(bc[:, NE:NE + 1], wgt[:])
            ps_bc = psumS.tile([CH, NE + 1], f32, tag="psb")
            nc.tensor.matmul(ps_bc[:], ones_sb[:], bc[:], start=True, stop=True)
            sc = spool.tile([CH, NE + 1], f32, tag="sc")
            nc.vector.tensor_copy(sc[:], ps_bc[:])

            # ---- expert-weight select: Wsel = sum_e onehot[e] * We[e] ----
            wA = selpool.tile([CH, 9 * CH], f32, tag="wA")
            nc.vector.tensor_scalar_mul(wA[:], we_sb[:, 0:1152], sc[:, 0:1])
            wB = selpool.tile([CH, 9 * CH], f32, tag="wB")
            nc.vector.scalar_tensor_tensor(wB[:], we_sb[:, 1152:2304], sc[:, 1:2],
                                           wA[:], op0=OP.mult, op1=OP.add)
            wS = selpool.tile([CH, 9 * CH], f32r, tag="wS")
            nc.vector.scalar_tensor_tensor(wS[:], we_sb[:, 2304:3456], sc[:, 2:3],
                                           wB[:], op0=OP.mult, op1=OP.add)
            bA = spool.tile([CH, 1], f32, tag="bA")
            nc.vector.tensor_scalar_mul(bA[:], be_sb[:, 0:1], sc[:, 0:1])
            bB = spool.tile([CH, 1], f32, tag="bB")
            nc.vector.scalar_tensor_tensor(bB[:], be_sb[:, 1:2], sc[:, 1:2],
                                           bA[:], op0=OP.mult, op1=OP.add)
            bS = spool.tile([CH, 1], f32, tag="bS")
            nc.vector.scalar_tensor_tensor(bS[:], be_sb[:, 2:3], sc[:, 2:3],
                                           bB[:], op0=OP.mult, op1=OP.add)

            # ---- shared expert 3x3 conv + SiLU ----
            for i in range(NT):
                ps = psum.tile([CH, TN], f32, tag="ps")
                conv_tile_matmuls(ps, ws_sb, i)
                act_silu(sh_sb[:, i * TN:(i + 1) * TN], ps[:], bs_sb[:])

            # ---- routed expert conv + moe combine + fused cv2 (1-tile lag) ----
            def cv2_tile(i):
                ft = fp3[:, i * RPT + 1: i * RPT + 1 + RPT, 1: 1 + W]
                for h in range(2):
                    po = psum.tile([CH, TN], f32, tag="ps")
                    nc.tensor.matmul(po[:], w2_sb[:, h * 128: h * 128 + 128],
                                     a_sb[:, i * TN:(i + 1) * TN],
                                     start=True, stop=False)
                    nc.tensor.matmul(po[:], w2_sb[:, 256 + h * 128: 256 + h * 128 + 128],
                                     ft, start=False, stop=False)
                    nc.tensor.matmul(po[:], w2_sb[:, 512 + h * 128: 512 + h * 128 + 128],
                                     moe_sb[:, i * TN:(i + 1) * TN],
                                     start=False, stop=True)
                    ot = opool.tile([CH, TN], f32, tag="ot")
                    act_silu(ot[:], po[:], b2_sb[:, h:h + 1])
                    nc.sync.dma_start(y_d[b, h, :, i * TN:(i + 1) * TN], ot[:])

            for i in range(NT):
                ps = psum.tile([CH, TN], f32, tag="ps")
                conv_tile_matmuls(ps, wS, i)
                rt = rpool.tile([CH, TN], f32, tag="rt")
                act_silu(rt[:], ps[:], bS[:])
                nc.vector.scalar_tensor_tensor(
                    moe_sb[:, i * TN:(i + 1) * TN], rt[:], sc[:, NE:NE + 1],
                    sh_sb[:, i * TN:(i + 1) * TN], op0=OP.mult, op1=OP.add)
                if i > 0:
                    cv2_tile(i - 1)
            cv2_tile(NT - 1)

    if reps == 1:
        _body()
    else:
        # HW timing mode: repeat the whole workload in a hardware loop
        # (same instruction count / compile cost; R x device work).
        with tc.For_i(0, reps, 1):
            _body()


def build(reps=1, sim_compat=False):
    from contextlib import ExitStack
    nc = bacc.Bacc("TRN2", target_bir_lowering=False, debug=False,
                   num_devices=NCORES)
    with tile.TileContext(nc) as tc:
        with ExitStack() as ctx:
            _emit(nc, tc, ctx, reps=reps, sim_compat=sim_compat)
    nc.compile()
    return nc


def round_f32r(a):
    """Round fp32 to the PE's fp32r format: 11 explicit mantissa bits
    (round-to-nearest-even), low 12 bits zero. The result is both a valid
    fp32 value and a valid fp32r bit pattern."""
    a = np.ascontiguousarray(np.asarray(a, np.float32))
    bits = a.view(np.uint32).astype(np.uint64)
    lsb = (bits >> 12) & 1
    r = (bits + 0x7FF + lsb) & 0xFFFFF000
    return r.astype(np.uint32).view(np.float32)


def marshal_inputs(x, w1, b1, wr, br, ws, bs, we, be, w2, b2):
    """Host-side (tiny) weight re-layouts into matmul-friendly forms."""
    asf = lambda a: np.ascontiguousarray(np.asarray(a, dtype=np.float32))
    x = round_f32r(x)
    w1t = asf(np.asarray(w1, np.float32).reshape(2 * CH, C1).T.reshape(2, CH, 2 * CH))
    b1r = asf(np.asarray(b1, np.float32).reshape(2, CH))
    wrs = asf(np.asarray(wr, np.float32) / NPIX)
    brr = asf(np.asarray(br, np.float32).reshape(1, NE))
    wst = asf(np.asarray(ws, np.float32).transpose(1, 2, 3, 0).reshape(CH, 9 * CH))
    bsr = asf(np.asarray(bs, np.float32).reshape(CH, 1))
    wet = asf(np.asarray(we, np.float32).transpose(0, 2, 3, 4, 1).reshape(NE, CH, 9 * CH))
    ber = asf(np.asarray(be, np.float32).T)
    w2t = asf(np.asarray(w2, np.float32).reshape(C2, 3 * CH).T.reshape(3, CH, C2))
    b2r = asf(np.asarray(b2, np.float32).reshape(2, CH))
    w1t = round_f32r(w1t)
    wst = round_f32r(wst)
    wet = round_f32r(wet)
    w2t = round_f32r(w2t)
    shared = dict(w1t=w1t, b1r=b1r, wrs=wrs, brr=brr, wst=wst, bsr=bsr,
                  wet=wet, ber=ber, w2t=w2t, b2r=b2r)
    xc = x.reshape(NCORES, BPC, 2, CH, NPIX)
    in_maps = [dict(shared, x=np.ascontiguousarray(xc[c])) for c in range(NCORES)]
    return in_maps


_CACHE = {}


def _get_nc():
    if "nc" not in _CACHE:
        _CACHE["nc"] = build(reps=1)
    return _CACHE["nc"]


def kernel(x, w1, b1, wr, br, ws, bs, we, be, w2, b2):
    nc = _get_nc()
    in_maps = marshal_inputs(x, w1, b1, wr, br, ws, bs, we, be, w2, b2)
    res = run_bass_kernel_spmd(nc, in_maps, list(range(NCORES)))
    y = np.stack([res.results[c]["y"] for c in range(NCORES)])
    return np.ascontiguousarray(y.reshape(B, C2, H, W))
